# revision 1
# baseline (speedup 1.0000x reference)
"""nn_BasicBlock GNN message-passing kernel for 8 Trainium2 NeuronCores.

Strategy (edge-parallel, segment-sharded):
  * Host: sort edges by destination (cur_idx); pack each segment's edges
    into fixed-size chunks of K=8 slots (padding slots duplicate a real
    edge of the same segment, which never changes a max). Segments are
    assigned to the 8 cores contiguously, balanced by chunk count.
  * Device (per core, via one shard_map program): gather slot edges,
    run in_linear (two Linear+ReLU), chunk-max (reshape+max, no scatter),
    then a second gather groups each segment's chunk-maxes (padded to K2
    with a zero dummy chunk) and maxes them -> agg rows for the core's
    segment range (zero-clamped exactly like the reference). out_linear
    runs on the core's agg slice; outputs concatenate across cores.
  * No scatter ops and no collectives are needed.
"""
import numpy as np
import jax
import jax.numpy as jnp
from jax.sharding import Mesh, PartitionSpec as P
from jax.experimental.shard_map import shard_map

N_CORES = 8
K_SLOT = 8          # slots per chunk (stage-1 reduce width)

_fn_cache = {}


def _host_prep(cur_idx, last_idx, m_cur):
    """Build per-core slot tables. Returns (slot_last, slot_cur, slots2, seg_per_core)."""
    e = cur_idx.shape[0]
    order = np.argsort(cur_idx, kind="stable")
    s_cur = cur_idx[order]
    s_last = last_idx[order]
    deg = np.bincount(cur_idx, minlength=m_cur)
    nchunk_seg = (deg + K_SLOT - 1) // K_SLOT          # chunks per segment
    k2 = max(1, int(nchunk_seg.max()))

    # assign contiguous segment ranges to cores, balanced by chunk count
    csum = np.cumsum(nchunk_seg)
    total = int(csum[-1])
    bounds = [0]
    for c in range(1, N_CORES):
        bounds.append(int(np.searchsorted(csum, total * c / N_CORES)))
    bounds.append(m_cur)
    seg_starts = np.array(bounds[:-1], np.int64)
    seg_ends = np.array(bounds[1:], np.int64)

    seg_edge_start = np.concatenate([[0], np.cumsum(deg)])  # [m+1]
    seg_chunk_start = np.concatenate([[0], csum])           # [m+1] global chunk id

    # global slot table: for each chunk slot, which sorted-edge position?
    # chunk j of segment s covers edges seg_edge_start[s]+8j .. +8j+7 (clamped,
    # padding repeats the segment's first edge)
    nchunks_total = total
    seg_of_chunk = np.repeat(np.arange(m_cur), nchunk_seg)                # [C]
    chunk_rank = np.arange(nchunks_total) - seg_chunk_start[seg_of_chunk]  # within-seg chunk no.
    base = seg_edge_start[seg_of_chunk] + chunk_rank * K_SLOT              # [C]
    offs = np.arange(K_SLOT)[None, :]
    pos = base[:, None] + offs                                             # [C, 8]
    limit = seg_edge_start[seg_of_chunk] + deg[seg_of_chunk]               # [C]
    pad_mask = pos >= limit[:, None]
    first_edge = seg_edge_start[seg_of_chunk]
    pos = np.where(pad_mask, first_edge[:, None], pos)                     # dup-pad

    slot_last = s_last[pos]            # [C, 8] gather ids into last_*
    slot_cur = s_cur[pos]              # [C, 8] segment ids (for B term)

    # per-core chunk ranges, padded to equal length (mult of CH for scan)
    core_cstart = seg_chunk_start[seg_starts]
    core_cend = seg_chunk_start[seg_ends]
    ncl = (core_cend - core_cstart).astype(np.int64)
    seg_per_core = (seg_ends - seg_starts).astype(np.int64)
    max_segs = int(seg_per_core.max())
    ncl_max = int(ncl.max())
    CH = 2048                                   # chunks per scan step
    ncl_pad = ((ncl_max + CH - 1) // CH) * CH

    sl = np.zeros((N_CORES, ncl_pad, K_SLOT), np.int32)
    sc = np.zeros((N_CORES, ncl_pad, K_SLOT), np.int32)
    slots2 = np.full((N_CORES, max_segs, k2), ncl_pad, np.int32)  # default -> dummy zero row
    for c in range(N_CORES):
        a, b = int(core_cstart[c]), int(core_cend[c])
        n = b - a
        sl[c, :n] = slot_last[a:b]
        sc[c, :n] = slot_cur[a:b]
        s0, s1 = int(seg_starts[c]), int(seg_ends[c])
        nseg = s1 - s0
        # local chunk ids for each segment's chunks
        st = (seg_chunk_start[s0:s1] - a).astype(np.int32)      # [nseg]
        cnt = nchunk_seg[s0:s1].astype(np.int32)
        k2g = np.arange(k2)[None, :]
        ids = st[:, None] + k2g
        ids = np.where(k2g < cnt[:, None], ids, ncl_pad)        # pad -> dummy
        slots2[c, :nseg] = ids
    return sl, sc, slots2, seg_per_core.astype(np.int32), ncl_pad, max_segs, k2, CH


def _build(m_cur, ncl_pad, max_segs, k2, CH, h_dim):
    devs = jax.devices()[:N_CORES]
    mesh = Mesh(np.array(devs), ("x",))
    nsteps = ncl_pad // CH

    def f(lc, lf, cc, sl, sc, slots2, W1, b1, W2, b2, W3, b3, W4, b4):
        sl2 = sl.reshape(nsteps, CH * K_SLOT)
        sc2 = sc.reshape(nsteps, CH * K_SLOT)

        # fold in_linear layer 1 into per-node tables:
        #   A[l] = [lf|lc][l] @ W1 + b1,  B[c] = cc[c] @ W1[64:]
        # so per edge  x1 = relu(A[l] - B[c])  (exact same math)
        A = lf @ W1[:lf.shape[1]] + lc @ W1[lf.shape[1]:] + b1
        B = cc @ W1[lf.shape[1]:]

        def body(carry, t):
            l, c = t
            x = jax.nn.relu(A[l] - B[c])
            x = jax.nn.relu(x @ W2 + b2)
            cm = x.reshape(CH, K_SLOT, h_dim).max(axis=1)
            return carry, cm

        _, cms = jax.lax.scan(body, 0, (sl2, sc2))
        chunkmax = cms.reshape(nsteps * CH, h_dim)
        chunkmax = jnp.concatenate([chunkmax, jnp.zeros((1, h_dim), jnp.float32)], axis=0)
        agg = chunkmax[slots2].max(axis=1)          # [max_segs, h]
        agg = jnp.maximum(agg, 0.0)
        y = jax.nn.relu(agg @ W3 + b3)
        y = jax.nn.relu(y @ W4 + b4)
        return y

    rep = P()
    return jax.jit(
        shard_map(
            f, mesh=mesh,
            in_specs=(rep, rep, rep, P("x"), P("x"), P("x"),
                      rep, rep, rep, rep, rep, rep, rep, rep),
            out_specs=P("x"),
            check_rep=False,
        )
    )


def kernel(last_coors, last_features, current_coors, edge,
           W1, b1, W2, b2, W3, b3, W4, b4):
    cur_idx = np.asarray(edge[0], dtype=np.int64)
    last_idx = np.asarray(edge[1], dtype=np.int64)
    m_cur = current_coors.shape[0]
    h_dim = np.asarray(W2).shape[1]

    sl, sc, slots2, seg_per_core, ncl_pad, max_segs, k2, CH = _host_prep(
        cur_idx, last_idx, m_cur)

    key = (m_cur, ncl_pad, max_segs, k2, CH, h_dim)
    if key not in _fn_cache:
        _fn_cache[key] = _build(*key)
    fn = _fn_cache[key]

    y = fn(jnp.asarray(np.asarray(last_coors), jnp.float32),
           jnp.asarray(np.asarray(last_features), jnp.float32),
           jnp.asarray(np.asarray(current_coors), jnp.float32),
           jnp.asarray(sl.reshape(N_CORES * ncl_pad, K_SLOT)),
           jnp.asarray(sc.reshape(N_CORES * ncl_pad, K_SLOT)),
           jnp.asarray(slots2.reshape(N_CORES * max_segs, k2)),
           jnp.asarray(np.asarray(W1), jnp.float32), jnp.asarray(np.asarray(b1), jnp.float32),
           jnp.asarray(np.asarray(W2), jnp.float32), jnp.asarray(np.asarray(b2), jnp.float32),
           jnp.asarray(np.asarray(W3), jnp.float32), jnp.asarray(np.asarray(b3), jnp.float32),
           jnp.asarray(np.asarray(W4), jnp.float32), jnp.asarray(np.asarray(b4), jnp.float32))
    y = np.asarray(jax.block_until_ready(y), dtype=np.float32)

    # stitch per-core segment slices back to [m_cur, h]
    out = np.zeros((m_cur, h_dim), np.float32)
    pos = 0
    row = 0
    for c in range(N_CORES):
        n = int(seg_per_core[c])
        out[pos:pos + n] = y[row:row + n]
        pos += n
        row += max_segs
    return out

